# revision 2
# baseline (speedup 1.0000x reference)
"""Trainium2 Bass kernel for ConvUnrolledISTAEncoder.

Math (per batch row b):
  h_x = Wx-conv(x) + bx            stride==kernel conv -> K=16 contraction matmul
  h   = relu(h_x); h = relu(h_x + Wh@h) x7          (8 ISTA iterations total)
  k   = softplus(Wk@h + bk) + eps
  th  = softplus(Wt@h + bt) + eps
  lg[b,a,t,c] = (Wp@h + bp)[3a+c, t]

Sharding: data-parallel over B -- 8 cores x 2 batch rows. Params replicated.

Per-core layout: windows n = b_loc*16384 + t (32768 per core). Host pre-arranges
x into xr[k, n] = x[b, 16t+k] so the conv is a single K=16 f32r matmul per tile.
h lives as [128, chunk] tiles (atoms on partitions); all matmuls f32r (1 cyc/col).

ISTA add+relu engine split (per chunk, to balance ACT/DVE/GPSIMD):
  - ACT-path iters: re-accumulate the conv matmul into the same PSUM bank as
    Wh@h (start=False), then one ACT pass relu(psum + bx).
  - DVE-path iters: DVE add (psum + hx) then GPSIMD relu (writes f32r).
softplus has no ACT table entry; computed as -ln(sigmoid(-(z+b))) + eps with
two ACT passes (Sigmoid scale=-1 bias=-b, then in-place Ln) + GPSIMD (*-1 +eps).
"""

import os
import sys

import numpy as np

for _p in ("/opt/trn_rl_repo", "/root/.axon_site/_ro/trn_rl_repo"):
    if os.path.isdir(_p) and _p not in sys.path:
        sys.path.insert(0, _p)

import concourse.bacc as bacc
import concourse.mybir as mybir
import concourse.tile as tile
from concourse.bass_utils import run_bass_kernel_spmd

F32 = mybir.dt.float32
F32R = mybir.dt.float32r
AF = mybir.ActivationFunctionType
ALU = mybir.AluOpType

B, T = 16, 262144
A, K, TP = 128, 16, 16384
NCORES = 8
BLOC = B // NCORES          # batch rows per core
NW = BLOC * TP              # windows per core
CH = 2048                   # windows per chunk
NCH = NW // CH              # chunks per core
NB = CH // 512              # psum banks per chunk tile
EPS = 1e-4
ACT_ITERS = (2, 5, 8)       # ISTA iters via ACT + psum re-accum; rest DVE+GPSIMD

_CACHE = {}


def _build():
    if "nc" in _CACHE:
        return _CACHE["nc"]
    nc = bacc.Bacc("TRN2", target_bir_lowering=False, debug=False, num_devices=NCORES)

    xr_d = nc.dram_tensor("xr", (K, NW), F32R, kind="ExternalInput")
    wx_d = nc.dram_tensor("wx", (K, A), F32R, kind="ExternalInput")
    wh_d = nc.dram_tensor("wh", (A, A), F32R, kind="ExternalInput")
    wk_d = nc.dram_tensor("wk", (A, A), F32R, kind="ExternalInput")
    wt_d = nc.dram_tensor("wt", (A, A), F32R, kind="ExternalInput")
    wp_d = nc.dram_tensor("wp", (A, 3, A), F32R, kind="ExternalInput")
    bias_d = nc.dram_tensor("bias", (A, 6), F32, kind="ExternalInput")
    ko_d = nc.dram_tensor("ko", (BLOC, A, TP), F32, kind="ExternalOutput")
    th_d = nc.dram_tensor("th", (BLOC, A, TP), F32, kind="ExternalOutput")
    lg_d = nc.dram_tensor("lg", (BLOC, A, TP, 3), F32, kind="ExternalOutput")

    with tile.TileContext(nc) as tc:
        with (
            tc.tile_pool(name="wpool", bufs=1) as wpool,
            tc.tile_pool(name="xpool", bufs=2) as xpool,
            tc.tile_pool(name="hxpool", bufs=2) as hxpool,
            tc.tile_pool(name="hpool", bufs=3) as hpool,
            tc.tile_pool(name="tpool", bufs=2) as tpool,
            tc.tile_pool(name="headpool", bufs=3) as headpool,
            tc.tile_pool(name="pipool", bufs=2) as pipool,
            tc.tile_pool(name="psum", bufs=2, space="PSUM") as pspool,
        ):
            wx_s = wpool.tile([K, A], F32R, tag="wx")
            wh_s = wpool.tile([A, A], F32R, tag="wh")
            wk_s = wpool.tile([A, A], F32R, tag="wk")
            wt_s = wpool.tile([A, A], F32R, tag="wt")
            wp_s = wpool.tile([A, 3, A], F32R, tag="wp")
            bias_s = wpool.tile([A, 6], F32, tag="bias")
            for dst, src in (
                (wx_s, wx_d), (wh_s, wh_d), (wk_s, wk_d),
                (wt_s, wt_d), (wp_s, wp_d), (bias_s, bias_d),
            ):
                nc.sync.dma_start(out=dst[:], in_=src[:])
            bx = bias_s[:, 0:1]
            nbk = bias_s[:, 1:2]          # -bk
            nbt = bias_s[:, 2:3]          # -bt
            bp = [bias_s[:, 3 + c : 4 + c] for c in range(3)]

            for ci in range(NCH):
                b, t0 = divmod(ci * CH, TP)
                xt = xpool.tile([K, CH], F32R, tag="xt")
                nc.sync.dma_start(out=xt[:], in_=xr_d[:, ci * CH : (ci + 1) * CH])

                pshx = pspool.tile([A, CH], F32, tag="mm")
                for j in range(NB):
                    s = slice(512 * j, 512 * (j + 1))
                    nc.tensor.matmul(pshx[:, s], wx_s[:], xt[:, s])
                hx = hxpool.tile([A, CH], F32, tag="hx")
                nc.scalar.activation(hx[:], pshx[:], AF.Identity, bias=bx)
                h = hpool.tile([A, CH], F32R, tag="h")
                nc.gpsimd.tensor_relu(h[:], hx[:])

                for it in range(2, 9):
                    psm = pspool.tile([A, CH], F32, tag="mm")
                    h2 = hpool.tile([A, CH], F32R, tag="h")
                    if it in ACT_ITERS:
                        for j in range(NB):
                            s = slice(512 * j, 512 * (j + 1))
                            nc.tensor.matmul(
                                psm[:, s], wh_s[:], h[:, s], start=True, stop=False
                            )
                            nc.tensor.matmul(
                                psm[:, s], wx_s[:], xt[:, s], start=False, stop=True
                            )
                        nc.scalar.activation(h2[:], psm[:], AF.Relu, bias=bx)
                    else:
                        for j in range(NB):
                            s = slice(512 * j, 512 * (j + 1))
                            nc.tensor.matmul(psm[:, s], wh_s[:], h[:, s])
                        tmp = tpool.tile([A, CH], F32, tag="tmp")
                        nc.vector.tensor_add(tmp[:], psm[:], hx[:])
                        nc.gpsimd.tensor_relu(h2[:], tmp[:])
                    h = h2

                for w_s, nbias_ap, out_d in (
                    (wk_s, nbk, ko_d),
                    (wt_s, nbt, th_d),
                ):
                    psk = pspool.tile([A, CH], F32, tag="mm")
                    for j in range(NB):
                        s = slice(512 * j, 512 * (j + 1))
                        nc.tensor.matmul(psk[:, s], w_s[:], h[:, s])
                    o = headpool.tile([A, CH], F32, tag="head")
                    # softplus(z+b) = -ln(sigmoid(-(z+b)))
                    nc.scalar.activation(
                        o[:], psk[:], AF.Sigmoid, bias=nbias_ap, scale=-1.0
                    )
                    nc.scalar.activation(o[:], o[:], AF.Ln)
                    nc.gpsimd.tensor_scalar(
                        out=o[:], in0=o[:], scalar1=-1.0, scalar2=EPS,
                        op0=ALU.mult, op1=ALU.add,
                    )
                    nc.sync.dma_start(out=out_d[b, :, t0 : t0 + CH], in_=o[:])

                pi = pipool.tile([A, CH, 3], F32, tag="pi")
                for c in range(3):
                    psp = pspool.tile([A, CH], F32, tag="mm")
                    for j in range(NB):
                        s = slice(512 * j, 512 * (j + 1))
                        nc.tensor.matmul(psp[:, s], wp_s[:, c, :], h[:, s])
                    nc.vector.tensor_scalar_add(pi[:, :, c], psp[:], bp[c])
                nc.sync.dma_start(out=lg_d[b, :, t0 : t0 + CH, :], in_=pi[:])

    nc.compile()
    _CACHE["nc"] = nc
    return nc


def kernel(x, Wx, bx, Wh, Wk, bk, Wt, bt, Wp, bp):
    x = np.asarray(x, dtype=np.float32)
    Wx = np.asarray(Wx, dtype=np.float32)
    bx = np.asarray(bx, dtype=np.float32)
    Wh = np.asarray(Wh, dtype=np.float32)
    Wk = np.asarray(Wk, dtype=np.float32)
    bk = np.asarray(bk, dtype=np.float32)
    Wt = np.asarray(Wt, dtype=np.float32)
    bt = np.asarray(bt, dtype=np.float32)
    Wp = np.asarray(Wp, dtype=np.float32)
    bp = np.asarray(bp, dtype=np.float32)

    nc = _build()

    wxT = np.ascontiguousarray(Wx[:, 0, :].T)                       # [16, 128]
    whT = np.ascontiguousarray(Wh.T)
    wkT = np.ascontiguousarray(Wk.T)
    wtT = np.ascontiguousarray(Wt.T)
    wpT = np.ascontiguousarray(Wp.reshape(A, 3, A).transpose(2, 1, 0))  # [i, c, a]
    bias = np.stack(
        [bx, -bk, -bt]
        + [np.ascontiguousarray(bp.reshape(A, 3)[:, c]) for c in range(3)],
        axis=1,
    ).astype(np.float32)                                            # [128, 6]

    in_maps = []
    for c in range(NCORES):
        xc = x[BLOC * c : BLOC * (c + 1), 0, :]                     # [BLOC, T]
        xr = np.ascontiguousarray(
            xc.reshape(BLOC, TP, K).transpose(2, 0, 1).reshape(K, NW)
        )
        in_maps.append(
            dict(xr=xr, wx=wxT, wh=whT, wk=wkT, wt=wtT, wp=wpT, bias=bias)
        )

    res = run_bass_kernel_spmd(
        nc,
        in_maps,
        list(range(NCORES)),
        trace=bool(int(os.environ.get("KTRACE", "0"))),
    )
    _CACHE["last_results"] = res

    ko = np.concatenate([res.results[c]["ko"] for c in range(NCORES)], axis=0)
    th = np.concatenate([res.results[c]["th"] for c in range(NCORES)], axis=0)
    lg = np.concatenate([res.results[c]["lg"] for c in range(NCORES)], axis=0)
    return ko, th, lg
